# revision 5
# baseline (speedup 1.0000x reference)
"""Bass/Tile TRN2 kernel for BitLinear causal self-attention (B=4, T=1024, C=1024, H=16).

Sharding: tensor-parallel over heads (2 heads/core, 8 cores) for qkv+attention,
then one AllToAll reshards y to row (token) shards for the output projection.
Per-core partial layernorm stats for the second BitLinear ride inside the
AllToAll payload (hi/lo bf16 split), so the kernel uses exactly one collective.
"""

import functools
import math
from contextlib import ExitStack

import ml_dtypes
import numpy as np

import concourse.bacc as bacc
import concourse.bass as bass
import concourse.mybir as mybir
import concourse.tile as tile
from concourse import masks as masks_mod
from concourse.bass_utils import run_bass_kernel_spmd

B, T, C = 4, 1024, 1024
H, HD = 16, 64
NCORES = 8
HPC = H // NCORES          # heads per core = 2
TOK = B * T                # 4096
RPC = TOK // NCORES        # token rows per core = 512
QB = 128.0
EPS = 1e-5

BF16 = mybir.dt.bfloat16
F32 = mybir.dt.float32
AF = mybir.ActivationFunctionType
ALU = mybir.AluOpType
AX = mybir.AxisListType

N_TB = TOK // 512          # 8 t-blocks of 512 for qkv
SHARD_ROWS = 130           # 128 y rows + stats hi row + stats lo row


def _emit(nc, tc, ctx):
    qxT = nc.dram_tensor("qxT", [C, TOK], BF16, kind="ExternalInput")
    qwinT = nc.dram_tensor("qwinT", [C, 3 * HPC * HD], BF16, kind="ExternalInput")
    qwoutT = nc.dram_tensor("qwoutT", [C, C], BF16, kind="ExternalInput")
    consts = nc.dram_tensor("consts", [1, 8], F32, kind="ExternalInput")
    bsel = nc.dram_tensor("bsel", [1, 4], F32, kind="ExternalInput")
    msel = nc.dram_tensor("msel", [1, 8], F32, kind="ExternalInput")
    out = nc.dram_tensor("out", [RPC, C], F32, kind="ExternalOutput")

    singles = ctx.enter_context(tc.tile_pool(name="singles", bufs=1))
    big = ctx.enter_context(tc.tile_pool(name="big", bufs=3, space="PSUM"))
    small = ctx.enter_context(tc.tile_pool(name="small", bufs=2, space="PSUM"))
    sb = ctx.enter_context(tc.tile_pool(name="sb", bufs=2))
    dram = ctx.enter_context(tc.tile_pool(name="dram", bufs=1, space="DRAM"))

    # ---- setup: identities, consts ----
    ident_bf = singles.tile([128, 128], BF16)
    masks_mod.make_identity(nc, ident_bf[:])
    ident_f32 = singles.tile([128, 128], F32)
    masks_mod.make_identity(nc, ident_f32[:])

    ones_row = singles.tile([1, 128], F32)
    nc.vector.memset(ones_row[:], 1.0)
    ones_col = singles.tile([128, 1], F32)
    nc.vector.memset(ones_col[:], 1.0)
    ones8 = singles.tile([8, 1], F32)
    nc.vector.memset(ones8[:], 1.0)

    csb = singles.tile([1, 8], F32)
    nc.sync.dma_start(csb[:], consts[:])
    bsel_sb = singles.tile([1, 4], F32)
    nc.sync.dma_start(bsel_sb[:], bsel[:])
    msel_sb = singles.tile([1, 8], F32)
    nc.sync.dma_start(msel_sb[:], msel[:])

    # broadcast att_scale to [128,1] for the exp scale argument
    cb_ps = small.tile([128, 8], F32, tag="small")
    nc.tensor.matmul(cb_ps[:], ones_row[:], csb[:])
    cbc = singles.tile([128, 8], F32)
    nc.vector.tensor_copy(cbc[:], cb_ps[:])

    # ---- resident weights ----
    qwin_sb = []
    for c in range(8):
        t_ = sb.tile([128, 3 * HPC * HD], BF16, tag=f"qwin{c}", bufs=1)
        nc.sync.dma_start(t_[:], qwinT[c * 128:(c + 1) * 128, :])
        qwin_sb.append(t_)
    qwout_sb = []
    for c in range(8):
        t_ = sb.tile([128, C], BF16, tag=f"qwout{c}", bufs=1)
        nc.sync.dma_start(t_[:], qwoutT[c * 128:(c + 1) * 128, :])
        qwout_sb.append(t_)

    qT_sb = singles.tile([128, TOK], BF16)
    kT_sb = singles.tile([128, TOK], BF16)
    vT_sb = singles.tile([128, TOK], BF16)

    # ---- phase 1: qkv projection ----
    for tb in range(N_TB):
        qx_t = []
        for c in range(8):
            t_ = sb.tile([128, 512], BF16, tag=f"qx{c}", bufs=2)
            nc.sync.dma_start(t_[:], qxT[c * 128:(c + 1) * 128, tb * 512:(tb + 1) * 512])
            qx_t.append(t_)
        qk_ps = big.tile([128, 1024], F32, tag="big")
        v_ps = big.tile([128, 512], F32, tag="big")
        for c in range(8):
            st, sp = (c == 0), (c == 7)
            nc.tensor.matmul(qk_ps[:, 0:512], qwin_sb[c][:, 0:128], qx_t[c][:], start=st, stop=sp)
            nc.tensor.matmul(qk_ps[:, 512:1024], qwin_sb[c][:, 128:256], qx_t[c][:], start=st, stop=sp)
            nc.tensor.matmul(v_ps[:], qwin_sb[c][:, 256:384], qx_t[c][:], start=st, stop=sp)
        nc.vector.tensor_copy(qT_sb[:, tb * 512:(tb + 1) * 512], qk_ps[:, 0:512])
        nc.vector.tensor_copy(kT_sb[:, tb * 512:(tb + 1) * 512], qk_ps[:, 512:1024])
        nc.vector.tensor_copy(vT_sb[:, tb * 512:(tb + 1) * 512], v_ps[:])

    # ---- phase 1b: v transpose to token-major v_aug tiles ----
    va = []  # per global 128-token block: [128, 130] (two heads x (64 v + ones col))
    for tb32 in range(32):
        t_ = singles.tile([128, 2 * (HD + 1)], BF16, tag=f"va{tb32}", name=f"va{tb32}")
        # ones columns at 64 and 129
        nc.vector.memset(t_[:, HD::HD + 1], 1.0)
        va.append(t_)
    for tb32 in range(32):
        tr_ps = small.tile([128, 128], BF16, tag="small")
        nc.tensor.transpose(tr_ps[:], vT_sb[:, tb32 * 128:(tb32 + 1) * 128], ident_bf[:])
        nc.vector.tensor_copy(va[tb32][:, 0:HD], tr_ps[:, 0:HD])
        nc.vector.tensor_copy(va[tb32][:, HD + 1:2 * HD + 1], tr_ps[:, HD:2 * HD])

    # ---- phase 2: attention per (head-local, batch) pair ----
    yT_sb = singles.tile([128, TOK], BF16)        # y^T: rows = local out cols, cols = tokens
    stats = singles.tile([128, 9], F32)           # cols 0-3 sum_b, 4-7 sumsq_b, 8 absmax
    sq_tmp = singles.tile([128, 512], BF16)       # scratch for tensor_tensor_reduce

    pair_idx = 0
    for hl in range(HPC):
        for b in range(B):
            qrow = hl * HD
            tbase = b * T
            # --- QK^T into S^T tiles, exp, causal mask ---
            se_tiles = {}  # (ib512, jbpair) -> sbuf tile [128,1024]
            for ib in range(2):
                jb_max = 4 * ib + 3  # crossing band ends at jb = 4*ib+3
                for jp in range(0, (jb_max + 1) // 2):
                    jb0, jb1 = 2 * jp, 2 * jp + 1
                    s_ps = big.tile([128, 1024], F32, tag="big", name=f"s_ps{pair_idx}_{ib}_{jp}")
                    for col, jb in ((0, jb0), (512, jb1)):
                        nc.tensor.matmul(
                            s_ps[:, col:col + 512],
                            kT_sb[qrow:qrow + HD, tbase + jb * 128:tbase + (jb + 1) * 128],
                            qT_sb[qrow:qrow + HD, tbase + ib * 512:tbase + (ib + 1) * 512],
                        )
                    se = sb.tile([128, 1024], BF16, tag="se", bufs=12, name=f"se{pair_idx}_{ib}_{jp}")
                    nc.scalar.activation(se[:], s_ps[:], AF.Exp, scale=cbc[:, 0:1])
                    # causal mask on diagonal-crossing tiles: keep j <= i - 128*p
                    for col, jb in ((0, jb0), (512, jb1)):
                        p = jb - 4 * ib
                        if 0 <= p <= 3:
                            nc.gpsimd.affine_select(
                                out=se[:, col:col + 512],
                                in_=se[:, col:col + 512],
                                compare_op=ALU.is_ge,
                                fill=0.0,
                                base=-128 * p,
                                pattern=[[1, 512]],
                                channel_multiplier=-1,
                            )
                    se_tiles[(ib, jp)] = se
            # --- PV with ones-column denominator ---
            y_pair = sb.tile([128, 512], BF16, tag="ypair", bufs=8, name=f"ypair{pair_idx}")
            for ib128 in range(8):
                ib512 = ib128 // 4
                icol = 128 * (ib128 % 4)
                pv_ps = small.tile([128, HD + 1], F32, tag="small", name=f"pv{pair_idx}_{ib128}")
                for jb in range(ib128 + 1):
                    se = se_tiles[(ib512, jb // 2)]
                    lhs = se[:, 512 * (jb % 2) + icol: 512 * (jb % 2) + icol + 128]
                    nc.tensor.matmul(
                        pv_ps[:], lhs, va[b * 8 + jb][:, (HD + 1) * hl:(HD + 1) * hl + HD + 1],
                        start=(jb == 0), stop=(jb == ib128),
                    )
                rec = sb.tile([128, 1], F32, tag="rec", bufs=2, name=f"rec{pair_idx}_{ib128}")
                nc.vector.reciprocal(rec[:], pv_ps[:, HD:HD + 1])
                nc.vector.tensor_scalar_mul(y_pair[:, ib128 * HD:(ib128 + 1) * HD], pv_ps[:, 0:HD], rec[:])
            # --- y transposes into yT ---
            for ib128 in range(8):
                ytr_ps = small.tile([HD, 128], BF16, tag="small", name=f"ytr{pair_idx}_{ib128}")
                nc.tensor.transpose(ytr_ps[:], y_pair[:, ib128 * HD:(ib128 + 1) * HD], ident_bf[:])
                nc.vector.tensor_copy(
                    yT_sb[hl * HD:(hl + 1) * HD, tbase + ib128 * 128:tbase + (ib128 + 1) * 128],
                    ytr_ps[:],
                )
            # --- stats ---
            s1 = sb.tile([128, 1], F32, tag="st1", bufs=2, name=f"s1_{pair_idx}")
            nc.vector.reduce_sum(s1[:], y_pair[:], axis=AX.X)
            s2 = sb.tile([128, 1], F32, tag="st2", bufs=2, name=f"s2_{pair_idx}")
            nc.scalar.activation(sq_tmp[:], y_pair[:], AF.Square, accum_out=s2[:])
            s3 = sb.tile([128, 1], F32, tag="st3", bufs=2, name=f"s3_{pair_idx}")
            nc.vector.reduce_max(s3[:], y_pair[:], axis=AX.X, apply_absolute_value=True)
            if hl == 0:
                nc.vector.tensor_copy(stats[:, b:b + 1], s1[:])
                nc.vector.tensor_copy(stats[:, 4 + b:5 + b], s2[:])
            else:
                nc.vector.tensor_add(stats[:, b:b + 1], stats[:, b:b + 1], s1[:])
                nc.vector.tensor_add(stats[:, 4 + b:5 + b], stats[:, 4 + b:5 + b], s2[:])
            if pair_idx == 0:
                nc.vector.tensor_copy(stats[:, 8:9], s3[:])
            else:
                nc.vector.tensor_max(stats[:, 8:9], stats[:, 8:9], s3[:])
            pair_idx += 1

    # ---- phase 3: partition-reduce stats, build A2A payload ----
    st_ps = small.tile([1, 9], F32, tag="small")
    nc.tensor.matmul(st_ps[:], ones_col[:], stats[:])  # column sums over partitions
    trm_ps = small.tile([1, 128], F32, tag="small")
    nc.tensor.transpose(trm_ps[:], stats[:, 8:9], ident_f32[:])
    gmax_l = singles.tile([1, 1], F32)
    nc.vector.reduce_max(gmax_l[:], trm_ps[:], axis=AX.X)

    srow = singles.tile([1, 512], F32)
    nc.vector.memset(srow[:], 0.0)
    nc.vector.tensor_copy(srow[:, 0:8], st_ps[:, 0:8])
    nc.vector.tensor_scalar_mul(srow[:, 8:16], msel_sb[:], gmax_l[:])
    hi_row = singles.tile([1, 512], BF16)
    nc.vector.memset(hi_row[:], 0.0)
    nc.vector.tensor_copy(hi_row[:, 0:16], srow[:, 0:16])
    hi_f = singles.tile([1, 16], F32)
    nc.vector.tensor_copy(hi_f[:], hi_row[:, 0:16])
    lo_row = singles.tile([1, 512], BF16)
    nc.vector.memset(lo_row[:], 0.0)
    nc.vector.tensor_sub(lo_row[:, 0:16], srow[:, 0:16], hi_f[:])

    a2a_in = dram.tile([NCORES * SHARD_ROWS, 512], BF16)
    a2a_out = dram.tile([NCORES * SHARD_ROWS, 512], BF16)
    for i in range(NCORES):
        base = SHARD_ROWS * i
        nc.sync.dma_start(a2a_in[base:base + 128, :], yT_sb[:, i * 512:(i + 1) * 512])
        nc.sync.dma_start(a2a_in[base + 128:base + 129, :], hi_row[:])
        nc.sync.dma_start(a2a_in[base + 129:base + 130, :], lo_row[:])
    nc.gpsimd.collective_compute(
        "AllToAll", ALU.bypass,
        replica_groups=[list(range(NCORES))],
        ins=[a2a_in.opt()], outs=[a2a_out.opt()],
    )

    # ---- phase 4: global stats, quantize, output projection ----
    qy = []
    for j in range(NCORES):
        t_ = sb.tile([128, 512], BF16, tag=f"qy{j}", bufs=1, name=f"qy{j}")
        nc.sync.dma_start(t_[:], a2a_out[SHARD_ROWS * j:SHARD_ROWS * j + 128, :])
        qy.append(t_)
    a2a_r = a2a_out.rearrange("(s r) t -> s r t", r=SHARD_ROWS)
    sr_hi = singles.tile([8, 16], BF16)
    nc.sync.dma_start(sr_hi[:], a2a_r[:, 128, 0:16])
    sr_lo = singles.tile([8, 16], BF16)
    nc.sync.dma_start(sr_lo[:], a2a_r[:, 129, 0:16])
    stats_f = singles.tile([8, 16], F32)
    nc.vector.tensor_add(stats_f[:], sr_hi[:], sr_lo[:])

    glob_ps = small.tile([1, 16], F32, tag="small")
    nc.tensor.matmul(glob_ps[:], ones8[:], stats_f[:])
    sc = singles.tile([1, 24], F32)  # scratch row: mu 0:4, msq 4:8, var 8:12, A 12:16, B 16:20
    inv_tc = 1.0 / float(T * C)
    nc.vector.tensor_scalar_mul(sc[:, 0:4], glob_ps[:, 0:4], inv_tc)
    nc.vector.tensor_scalar_mul(sc[:, 4:8], glob_ps[:, 4:8], inv_tc)
    gmax = singles.tile([1, 1], F32)
    nc.vector.reduce_max(gmax[:], glob_ps[:, 8:16], axis=AX.X)
    nc.vector.tensor_mul(sc[:, 8:12], sc[:, 0:4], sc[:, 0:4])      # mu^2
    nc.vector.tensor_sub(sc[:, 8:12], sc[:, 4:8], sc[:, 8:12])     # var = msq - mu^2
    nc.vector.tensor_scalar_add(sc[:, 8:12], sc[:, 8:12], 1e-5)    # var + eps
    sig = singles.tile([1, 4], F32)
    nc.scalar.activation(sig[:], sc[:, 8:12], AF.Sqrt)
    rsig = singles.tile([1, 4], F32)
    nc.vector.reciprocal(rsig[:], sig[:])
    nc.vector.tensor_scalar_mul(sc[:, 12:16], rsig[:], csb[:, 1:2])  # A_b = beta2 / sigma_b
    nc.vector.tensor_mul(sc[:, 16:20], sc[:, 0:4], sc[:, 12:16])     # B_b = mu_b * A_b
    # select this core's batch scalars, pack (A, B, -bound, bound)
    row4 = singles.tile([1, 4], F32)
    tsel = singles.tile([1, 4], F32)
    nc.vector.tensor_mul(tsel[:], sc[:, 12:16], bsel_sb[:])
    nc.vector.reduce_sum(row4[:, 0:1], tsel[:], axis=AX.X)
    tsel2 = singles.tile([1, 4], F32)
    nc.vector.tensor_mul(tsel2[:], sc[:, 16:20], bsel_sb[:])
    nc.vector.reduce_sum(row4[:, 1:2], tsel2[:], axis=AX.X)
    nc.vector.tensor_scalar_mul(row4[:, 3:4], gmax[:], csb[:, 2:3])  # bound = cb' * gmax
    nc.vector.tensor_scalar_mul(row4[:, 2:3], row4[:, 3:4], -1.0)
    qsc_ps = small.tile([128, 4], F32, tag="small")
    nc.tensor.matmul(qsc_ps[:], ones_row[:], row4[:])
    qsc = singles.tile([128, 4], F32)
    nc.vector.tensor_copy(qsc[:], qsc_ps[:])

    for j in range(NCORES):
        nc.vector.tensor_scalar(
            out=qy[j][:], in0=qy[j][:], scalar1=qsc[:, 0:1], scalar2=qsc[:, 1:2],
            op0=ALU.mult, op1=ALU.subtract,
        )
        nc.vector.tensor_scalar(
            out=qy[j][:], in0=qy[j][:], scalar1=qsc[:, 2:3], scalar2=qsc[:, 3:4],
            op0=ALU.max, op1=ALU.min,
        )

    for tch in range(4):
        for oh in range(2):
            o_ps = big.tile([128, 512], F32, tag="big", name=f"ops{tch}_{oh}")
            for cj in range(8):
                nc.tensor.matmul(
                    o_ps[:], qy[cj][:, tch * 128:(tch + 1) * 128],
                    qwout_sb[cj][:, oh * 512:(oh + 1) * 512],
                    start=(cj == 0), stop=(cj == 7),
                )
            osb = sb.tile([128, 512], F32, tag="ob", bufs=2, name=f"osb{tch}_{oh}")
            nc.vector.tensor_copy(osb[:], o_ps[:])
            nc.sync.dma_start(out[tch * 128:(tch + 1) * 128, oh * 512:(oh + 1) * 512], osb[:])


@functools.lru_cache(maxsize=1)
def build():
    nc = bacc.Bacc(None)
    with tile.TileContext(nc) as tc:
        with ExitStack() as ctx:
            _emit(nc, tc, ctx)
    nc.finalize()
    return nc


def _host_prep(x, w_in, w_out):
    x = np.asarray(x, np.float32)
    w_in = np.asarray(w_in, np.float32)
    w_out = np.asarray(w_out, np.float32)

    a1 = w_in.mean()
    qw1 = np.sign(w_in - a1).astype(np.float32)
    b1 = np.abs(w_in).mean()
    a2 = w_out.mean()
    qw2 = np.sign(w_out - a2).astype(np.float32)
    b2 = np.abs(w_out).mean()

    mu = x.mean(axis=(1, 2), keepdims=True)
    var = x.var(axis=(1, 2), keepdims=True)
    g1 = np.abs(x).max()
    xn = (x - mu) / np.sqrt(var + 1e-5)
    qx = np.clip(xn * (QB / g1), -QB + EPS, QB - EPS)
    scale1 = b1 * g1 / QB

    bf = ml_dtypes.bfloat16
    qxT = np.ascontiguousarray(qx.reshape(TOK, C).T).astype(bf)
    qwoutT = np.ascontiguousarray(qw2.T).astype(bf)
    att_scale = scale1 * scale1 / math.sqrt(HD)
    cbound = (QB - EPS) / QB * b2 * scale1
    consts = np.array([[att_scale, b2, cbound, 0, 0, 0, 0, 0]], np.float32)

    in_maps = []
    for core in range(NCORES):
        r0 = core * 128
        qwin = np.concatenate(
            [qw1[r0:r0 + 128], qw1[C + r0:C + r0 + 128], qw1[2 * C + r0:2 * C + r0 + 128]], axis=0
        )
        qwinT = np.ascontiguousarray(qwin.T).astype(bf)
        bsel_ = np.zeros((1, 4), np.float32)
        bsel_[0, core // 2] = 1.0
        msel_ = np.zeros((1, 8), np.float32)
        msel_[0, core] = 1.0
        in_maps.append({
            "qxT": qxT, "qwinT": qwinT, "qwoutT": qwoutT,
            "consts": consts, "bsel": bsel_, "msel": msel_,
        })
    return in_maps


def kernel(x, w_in, w_out):
    in_maps = _host_prep(x, w_in, w_out)
    nc = build()
    res = run_bass_kernel_spmd(nc, in_maps, core_ids=list(range(NCORES)))
    out = np.concatenate([np.asarray(res.results[i]["out"]) for i in range(NCORES)], axis=0)
    return out.reshape(B, T, C).astype(np.float32)


# revision 8
# speedup vs baseline: 1.0714x; 1.0714x over previous
"""Bass/Tile TRN2 kernel for BitLinear causal self-attention (B=4, T=1024, C=1024, H=16).

Sharding: tensor-parallel over heads (2 heads/core, 8 cores) for qkv+attention,
then AllToAll reshards y to row (token) shards for the output projection.
The AllToAll is split in two by head-half so the first half overlaps the
second half of attention; per-core partial layernorm stats for the second
BitLinear ride inside the second AllToAll payload (hi/lo bf16 split).
"""

import functools
import math
from contextlib import ExitStack

import ml_dtypes
import numpy as np

import concourse.bacc as bacc
import concourse.bass as bass
import concourse.mybir as mybir
import concourse.tile as tile
from concourse import masks as masks_mod
from concourse.bass_utils import run_bass_kernel_spmd

B, T, C = 4, 1024, 1024
H, HD = 16, 64
NCORES = 8
HPC = H // NCORES          # heads per core = 2
TOK = B * T                # 4096
RPC = TOK // NCORES        # token rows per core = 512
QB = 128.0
EPS = 1e-5

BF16 = mybir.dt.bfloat16
F32 = mybir.dt.float32
AF = mybir.ActivationFunctionType
ALU = mybir.AluOpType
AX = mybir.AxisListType

N_TB = TOK // 512          # 8 t-blocks of 512 for qkv


def _emit(nc, tc, ctx):
    qxT = nc.dram_tensor("qxT", [C, TOK], BF16, kind="ExternalInput")
    qwinT = nc.dram_tensor("qwinT", [C, 3 * HPC * HD], BF16, kind="ExternalInput")
    qwoutT = nc.dram_tensor("qwoutT", [C, C], BF16, kind="ExternalInput")
    consts = nc.dram_tensor("consts", [1, 8], F32, kind="ExternalInput")
    bsel = nc.dram_tensor("bsel", [1, 4], F32, kind="ExternalInput")
    msel = nc.dram_tensor("msel", [1, 8], F32, kind="ExternalInput")
    out = nc.dram_tensor("out", [RPC, C], F32, kind="ExternalOutput")

    singles = ctx.enter_context(tc.tile_pool(name="singles", bufs=1))
    big = ctx.enter_context(tc.tile_pool(name="big", bufs=3, space="PSUM"))
    small = ctx.enter_context(tc.tile_pool(name="small", bufs=2, space="PSUM"))
    sb = ctx.enter_context(tc.tile_pool(name="sb", bufs=2))
    dram = ctx.enter_context(tc.tile_pool(name="dram", bufs=1, space="DRAM"))

    # ---- setup: identities, consts ----
    ident_bf = singles.tile([128, 128], BF16)
    masks_mod.make_identity(nc, ident_bf[:])
    ident_f32 = singles.tile([128, 128], F32)
    masks_mod.make_identity(nc, ident_f32[:])

    ones_row = singles.tile([1, 128], F32)
    nc.vector.memset(ones_row[:], 1.0)
    ones_col = singles.tile([128, 1], F32)
    nc.vector.memset(ones_col[:], 1.0)
    ones8 = singles.tile([8, 1], F32)
    nc.vector.memset(ones8[:], 1.0)

    csb = singles.tile([1, 8], F32)
    nc.sync.dma_start(csb[:], consts[:])
    bsel_sb = singles.tile([1, 4], F32)
    nc.sync.dma_start(bsel_sb[:], bsel[:])
    msel_sb = singles.tile([1, 8], F32)
    nc.sync.dma_start(msel_sb[:], msel[:])

    # broadcast att_scale to [128,1] for the exp scale argument
    cb_ps = small.tile([128, 8], F32, tag="small")
    nc.tensor.matmul(cb_ps[:], ones_row[:], csb[:])
    cbc = singles.tile([128, 8], F32)
    nc.vector.tensor_copy(cbc[:], cb_ps[:])

    # ---- resident weights (single merged DMAs) ----
    qwin_all = singles.tile([128, 8 * 384], BF16)
    nc.sync.dma_start(qwin_all[:], qwinT.rearrange("(c p) o -> p c o", p=128))
    qwout_all = singles.tile([128, 8 * 1024], BF16)
    nc.sync.dma_start(qwout_all[:], qwoutT.rearrange("(c p) o -> p c o", p=128))

    def qwin(c, lo, hi):
        return qwin_all[:, c * 384 + lo:c * 384 + hi]

    def qwout(c, lo, hi):
        return qwout_all[:, c * 1024 + lo:c * 1024 + hi]

    qT_sb = singles.tile([128, TOK], BF16)
    kT_sb = singles.tile([128, TOK], BF16)
    vT_sb = singles.tile([128, TOK], BF16)

    qxT_r = qxT.rearrange("(c p) t -> p c t", p=128)

    # ---- phase 1: qkv projection ----
    for tb in range(N_TB):
        qx_tb = sb.tile([128, 8, 512], BF16, tag="qx", bufs=2, name=f"qx{tb}")
        nc.sync.dma_start(qx_tb[:], qxT_r[:, :, tb * 512:(tb + 1) * 512])
        qk_ps = big.tile([128, 1024], F32, tag="big", name=f"qkps{tb}")
        v_ps = big.tile([128, 512], F32, tag="big", name=f"vps{tb}")
        for c in range(8):
            st, sp = (c == 0), (c == 7)
            nc.tensor.matmul(qk_ps[:, 0:512], qwin(c, 0, 128), qx_tb[:, c, :], start=st, stop=sp)
            nc.tensor.matmul(qk_ps[:, 512:1024], qwin(c, 128, 256), qx_tb[:, c, :], start=st, stop=sp)
            nc.tensor.matmul(v_ps[:], qwin(c, 256, 384), qx_tb[:, c, :], start=st, stop=sp)
        nc.vector.tensor_copy(qT_sb[:, tb * 512:(tb + 1) * 512], qk_ps[:, 0:512])
        nc.vector.tensor_copy(kT_sb[:, tb * 512:(tb + 1) * 512], qk_ps[:, 512:1024])
        nc.vector.tensor_copy(vT_sb[:, tb * 512:(tb + 1) * 512], v_ps[:])

    # ---- phase 1b: v transpose to token-major v_aug tiles ----
    va = []  # per global 128-token block: [128, 130] (two heads x (64 v + ones col))
    for tb32 in range(32):
        t_ = singles.tile([128, 2 * (HD + 1)], BF16, tag=f"va{tb32}", name=f"va{tb32}")
        nc.vector.memset(t_[:, HD:HD + 1], 1.0)
        nc.vector.memset(t_[:, 2 * HD + 1:2 * HD + 2], 1.0)
        va.append(t_)
    for tb32 in range(32):
        tr_ps = small.tile([128, 128], BF16, tag="small")
        nc.tensor.transpose(tr_ps[:], vT_sb[:, tb32 * 128:(tb32 + 1) * 128], ident_bf[:])
        nc.vector.tensor_copy(va[tb32][:, 0:HD], tr_ps[:, 0:HD])
        nc.vector.tensor_copy(va[tb32][:, HD + 1:2 * HD + 1], tr_ps[:, HD:2 * HD])

    # ---- A2A buffers (split by head-half) ----
    # a2a1: blocks [64, 512]  = y^T rows of head-local 0
    # a2a2: blocks [66, 512]  = y^T rows of head-local 1 + stats hi/lo rows
    a2a1_in = dram.tile([NCORES * 64, 512], BF16)
    a2a1_out = dram.tile([NCORES * 64, 512], BF16)
    a2a2_in = dram.tile([NCORES * 66, 512], BF16)
    a2a2_out = dram.tile([NCORES * 66, 512], BF16)
    a2a1_in_r = a2a1_in.rearrange("(bb h p) t -> p bb h t", p=64, h=2)   # [64, 4, 2, 512]
    a2a2_in_r = a2a2_in.rearrange("(j p) t -> p j t", p=66)              # [66, 8, 512]

    # ---- phase 2: attention per (head-local, batch) pair ----
    yT_sb = singles.tile([128, TOK], BF16)        # y^T: rows = local out cols, cols = tokens
    stats = singles.tile([128, 9], F32)           # cols 0-3 sum_b, 4-7 sumsq_b, 8 absmax
    sq_tmp = singles.tile([128, 512], BF16)       # scratch for Square+accum

    pair_idx = 0
    for hl in range(HPC):
        for b in range(B):
            qrow = hl * HD
            tbase = b * T
            # --- QK^T into S^T tiles, exp, causal mask ---
            se_tiles = {}  # (ib512, jbpair) -> sbuf tile [128,1024]
            for ib in range(2):
                jb_max = 4 * ib + 3
                for jp in range(0, (jb_max + 1) // 2):
                    jb0, jb1 = 2 * jp, 2 * jp + 1
                    s_ps = big.tile([128, 1024], F32, tag="big", name=f"s_ps{pair_idx}_{ib}_{jp}")
                    for col, jb in ((0, jb0), (512, jb1)):
                        nc.tensor.matmul(
                            s_ps[:, col:col + 512],
                            kT_sb[qrow:qrow + HD, tbase + jb * 128:tbase + (jb + 1) * 128],
                            qT_sb[qrow:qrow + HD, tbase + ib * 512:tbase + (ib + 1) * 512],
                        )
                    se = sb.tile([128, 1024], BF16, tag="se", bufs=12, name=f"se{pair_idx}_{ib}_{jp}")
                    nc.scalar.activation(se[:], s_ps[:], AF.Exp, scale=cbc[:, 0:1])
                    for col, jb in ((0, jb0), (512, jb1)):
                        p = jb - 4 * ib
                        if 0 <= p <= 3:
                            nc.gpsimd.affine_select(
                                out=se[:, col:col + 512],
                                in_=se[:, col:col + 512],
                                compare_op=ALU.is_ge,
                                fill=0.0,
                                base=-128 * p,
                                pattern=[[1, 512]],
                                channel_multiplier=-1,
                            )
                    se_tiles[(ib, jp)] = se
            # --- PV with ones-column denominator ---
            y_pair = sb.tile([128, 512], BF16, tag="ypair", bufs=8, name=f"ypair{pair_idx}")
            for ib128 in range(8):
                ib512 = ib128 // 4
                icol = 128 * (ib128 % 4)
                pv_ps = small.tile([128, HD + 1], F32, tag="small", name=f"pv{pair_idx}_{ib128}")
                for jb in range(ib128 + 1):
                    se = se_tiles[(ib512, jb // 2)]
                    lhs = se[:, 512 * (jb % 2) + icol: 512 * (jb % 2) + icol + 128]
                    nc.tensor.matmul(
                        pv_ps[:], lhs, va[b * 8 + jb][:, (HD + 1) * hl:(HD + 1) * hl + HD + 1],
                        start=(jb == 0), stop=(jb == ib128),
                    )
                rec = sb.tile([128, 1], F32, tag="rec", bufs=2, name=f"rec{pair_idx}_{ib128}")
                nc.vector.reciprocal(rec[:], pv_ps[:, HD:HD + 1])
                nc.vector.tensor_scalar_mul(y_pair[:, ib128 * HD:(ib128 + 1) * HD], pv_ps[:, 0:HD], rec[:])
            # --- y transposes into yT ---
            for ib128 in range(8):
                ytr_ps = small.tile([HD, 128], BF16, tag="small", name=f"ytr{pair_idx}_{ib128}")
                nc.tensor.transpose(ytr_ps[:], y_pair[:, ib128 * HD:(ib128 + 1) * HD], ident_bf[:])
                nc.vector.tensor_copy(
                    yT_sb[hl * HD:(hl + 1) * HD, tbase + ib128 * 128:tbase + (ib128 + 1) * 128],
                    ytr_ps[:],
                )
            # --- stage this batch's y rows into the A2A input buffers ---
            if hl == 0:
                nc.sync.dma_start(a2a1_in_r[:, b, :, :], yT_sb[0:64, tbase:tbase + T])
            else:
                nc.sync.dma_start(
                    a2a2_in_r[0:64, 2 * b:2 * b + 2, :], yT_sb[64:128, tbase:tbase + T]
                )
            # --- stats ---
            s1 = sb.tile([128, 1], F32, tag="st1", bufs=2, name=f"s1_{pair_idx}")
            nc.vector.reduce_sum(s1[:], y_pair[:], axis=AX.X)
            s2 = sb.tile([128, 1], F32, tag="st2", bufs=2, name=f"s2_{pair_idx}")
            nc.scalar.activation(sq_tmp[:], y_pair[:], AF.Square, accum_out=s2[:])
            s3 = sb.tile([128, 1], F32, tag="st3", bufs=2, name=f"s3_{pair_idx}")
            nc.vector.reduce_max(s3[:], y_pair[:], axis=AX.X, apply_absolute_value=True)
            if hl == 0:
                nc.vector.tensor_copy(stats[:, b:b + 1], s1[:])
                nc.vector.tensor_copy(stats[:, 4 + b:5 + b], s2[:])
            else:
                nc.vector.tensor_add(stats[:, b:b + 1], stats[:, b:b + 1], s1[:])
                nc.vector.tensor_add(stats[:, 4 + b:5 + b], stats[:, 4 + b:5 + b], s2[:])
            if pair_idx == 0:
                nc.vector.tensor_copy(stats[:, 8:9], s3[:])
            else:
                nc.vector.tensor_max(stats[:, 8:9], stats[:, 8:9], s3[:])
            pair_idx += 1
        if hl == 0:
            # first half of y is staged: overlap this collective with hl=1 attention
            nc.gpsimd.collective_compute(
                "AllToAll", ALU.bypass,
                replica_groups=[list(range(NCORES))],
                ins=[a2a1_in.opt()], outs=[a2a1_out.opt()],
            )

    # ---- phase 3: partition-reduce stats, stage stats rows, second A2A ----
    st_ps = small.tile([1, 9], F32, tag="small")
    nc.tensor.matmul(st_ps[:], ones_col[:], stats[:])
    trm_ps = small.tile([1, 128], F32, tag="small")
    nc.tensor.transpose(trm_ps[:], stats[:, 8:9], ident_f32[:])
    gmax_l = singles.tile([1, 1], F32)
    nc.vector.reduce_max(gmax_l[:], trm_ps[:], axis=AX.X)

    srow = singles.tile([1, 512], F32)
    nc.vector.memset(srow[:], 0.0)
    nc.vector.tensor_copy(srow[:, 0:8], st_ps[:, 0:8])
    nc.vector.tensor_scalar_mul(srow[:, 8:16], msel_sb[:], gmax_l[:])
    # replicate the stats row to 8 partitions (one per dest), hi/lo bf16 split
    srep_ps = small.tile([8, 512], F32, tag="small")
    nc.tensor.matmul(srep_ps[:], ones_row[:, 0:8], srow[:])
    srep = singles.tile([8, 512], F32)
    nc.vector.tensor_copy(srep[:], srep_ps[:])
    hi8 = singles.tile([8, 512], BF16)
    nc.vector.tensor_copy(hi8[:], srep[:])
    hi8f = singles.tile([8, 512], F32)
    nc.vector.tensor_copy(hi8f[:], hi8[:])
    lo8 = singles.tile([8, 512], BF16)
    nc.vector.tensor_sub(lo8[:], srep[:], hi8f[:])
    # stage stats rows: dest rows 66j+64 (hi), 66j+65 (lo)
    nc.sync.dma_start(a2a2_in_r[64, :, :], hi8[:])
    nc.sync.dma_start(a2a2_in_r[65, :, :], lo8[:])
    nc.gpsimd.collective_compute(
        "AllToAll", ALU.bypass,
        replica_groups=[list(range(NCORES))],
        ins=[a2a2_in.opt()], outs=[a2a2_out.opt()],
    )

    # ---- phase 4: global stats, quantize, output projection ----
    qy = singles.tile([128, 8, 512], BF16)
    a2a1_o_r = a2a1_out.rearrange("(j p) t -> p j t", p=64)
    a2a2_o_r = a2a2_out.rearrange("(j p) t -> p j t", p=66)
    nc.sync.dma_start(qy[0:64, :, :], a2a1_o_r[:, :, :])
    nc.sync.dma_start(qy[64:128, :, :], a2a2_o_r[0:64, :, :])
    sr_hi = singles.tile([8, 16], BF16)
    nc.sync.dma_start(sr_hi[:], a2a2_o_r[64, :, 0:16])
    sr_lo = singles.tile([8, 16], BF16)
    nc.sync.dma_start(sr_lo[:], a2a2_o_r[65, :, 0:16])
    stats_f = singles.tile([8, 16], F32)
    nc.vector.tensor_add(stats_f[:], sr_hi[:], sr_lo[:])

    glob_ps = small.tile([1, 16], F32, tag="small")
    nc.tensor.matmul(glob_ps[:], ones8[:], stats_f[:])
    sc = singles.tile([1, 24], F32)  # mu 0:4, msq 4:8, var 8:12, A 12:16, B 16:20
    inv_tc = 1.0 / float(T * C)
    nc.vector.tensor_scalar_mul(sc[:, 0:4], glob_ps[:, 0:4], inv_tc)
    nc.vector.tensor_scalar_mul(sc[:, 4:8], glob_ps[:, 4:8], inv_tc)
    gmax = singles.tile([1, 1], F32)
    nc.vector.reduce_max(gmax[:], glob_ps[:, 8:16], axis=AX.X)
    nc.vector.tensor_mul(sc[:, 8:12], sc[:, 0:4], sc[:, 0:4])
    nc.vector.tensor_sub(sc[:, 8:12], sc[:, 4:8], sc[:, 8:12])
    nc.vector.tensor_scalar_add(sc[:, 8:12], sc[:, 8:12], 1e-5)
    sig = singles.tile([1, 4], F32)
    nc.scalar.activation(sig[:], sc[:, 8:12], AF.Sqrt)
    rsig = singles.tile([1, 4], F32)
    nc.vector.reciprocal(rsig[:], sig[:])
    nc.vector.tensor_scalar_mul(sc[:, 12:16], rsig[:], csb[:, 1:2])
    nc.vector.tensor_mul(sc[:, 16:20], sc[:, 0:4], sc[:, 12:16])
    row4 = singles.tile([1, 4], F32)
    tsel = singles.tile([1, 4], F32)
    nc.vector.tensor_mul(tsel[:], sc[:, 12:16], bsel_sb[:])
    nc.vector.reduce_sum(row4[:, 0:1], tsel[:], axis=AX.X)
    tsel2 = singles.tile([1, 4], F32)
    nc.vector.tensor_mul(tsel2[:], sc[:, 16:20], bsel_sb[:])
    nc.vector.reduce_sum(row4[:, 1:2], tsel2[:], axis=AX.X)
    nc.vector.tensor_scalar_mul(row4[:, 3:4], gmax[:], csb[:, 2:3])
    nc.vector.tensor_scalar_mul(row4[:, 2:3], row4[:, 3:4], -1.0)
    qsc_ps = small.tile([128, 4], F32, tag="small")
    nc.tensor.matmul(qsc_ps[:], ones_row[:], row4[:])
    qsc = singles.tile([128, 4], F32)
    nc.vector.tensor_copy(qsc[:], qsc_ps[:])

    qy_flat = qy.rearrange("p j t -> p (j t)")
    nc.vector.tensor_scalar(
        out=qy_flat, in0=qy_flat, scalar1=qsc[:, 0:1], scalar2=qsc[:, 1:2],
        op0=ALU.mult, op1=ALU.subtract,
    )
    nc.vector.tensor_scalar(
        out=qy_flat, in0=qy_flat, scalar1=qsc[:, 2:3], scalar2=qsc[:, 3:4],
        op0=ALU.max, op1=ALU.min,
    )

    for tch in range(4):
        osb = sb.tile([128, 1024], F32, tag="ob", bufs=2, name=f"osb{tch}")
        for oh in range(2):
            o_ps = big.tile([128, 512], F32, tag="big", name=f"ops{tch}_{oh}")
            for cj in range(8):
                nc.tensor.matmul(
                    o_ps[:], qy[:, cj, tch * 128:(tch + 1) * 128],
                    qwout(cj, oh * 512, (oh + 1) * 512),
                    start=(cj == 0), stop=(cj == 7),
                )
            nc.vector.tensor_copy(osb[:, oh * 512:(oh + 1) * 512], o_ps[:])
        nc.sync.dma_start(out[tch * 128:(tch + 1) * 128, :], osb[:])


@functools.lru_cache(maxsize=1)
def build():
    nc = bacc.Bacc(None)
    with tile.TileContext(nc) as tc:
        with ExitStack() as ctx:
            _emit(nc, tc, ctx)
    nc.finalize()
    return nc


def _host_prep(x, w_in, w_out):
    x = np.asarray(x, np.float32)
    w_in = np.asarray(w_in, np.float32)
    w_out = np.asarray(w_out, np.float32)

    a1 = w_in.mean()
    qw1 = np.sign(w_in - a1).astype(np.float32)
    b1 = np.abs(w_in).mean()
    a2 = w_out.mean()
    qw2 = np.sign(w_out - a2).astype(np.float32)
    b2 = np.abs(w_out).mean()

    mu = x.mean(axis=(1, 2), keepdims=True)
    var = x.var(axis=(1, 2), keepdims=True)
    g1 = np.abs(x).max()
    xn = (x - mu) / np.sqrt(var + 1e-5)
    qx = np.clip(xn * (QB / g1), -QB + EPS, QB - EPS)
    scale1 = b1 * g1 / QB

    bf = ml_dtypes.bfloat16
    qxT = np.ascontiguousarray(qx.reshape(TOK, C).T).astype(bf)
    qwoutT = np.ascontiguousarray(qw2.T).astype(bf)
    att_scale = scale1 * scale1 / math.sqrt(HD)
    cbound = (QB - EPS) / QB * b2 * scale1
    consts = np.array([[att_scale, b2, cbound, 0, 0, 0, 0, 0]], np.float32)

    in_maps = []
    for core in range(NCORES):
        r0 = core * 128
        qwin = np.concatenate(
            [qw1[r0:r0 + 128], qw1[C + r0:C + r0 + 128], qw1[2 * C + r0:2 * C + r0 + 128]], axis=0
        )
        qwinT = np.ascontiguousarray(qwin.T).astype(bf)
        bsel_ = np.zeros((1, 4), np.float32)
        bsel_[0, core // 2] = 1.0
        msel_ = np.zeros((1, 8), np.float32)
        msel_[0, core] = 1.0
        in_maps.append({
            "qxT": qxT, "qwinT": qwinT, "qwoutT": qwoutT,
            "consts": consts, "bsel": bsel_, "msel": msel_,
        })
    return in_maps


def kernel(x, w_in, w_out):
    in_maps = _host_prep(x, w_in, w_out)
    nc = build()
    res = run_bass_kernel_spmd(nc, in_maps, core_ids=list(range(NCORES)))
    out = np.concatenate([np.asarray(res.results[i]["out"]) for i in range(NCORES)], axis=0)
    return out.reshape(B, T, C).astype(np.float32)


# revision 10
# speedup vs baseline: 1.1474x; 1.0710x over previous
"""Bass/Tile TRN2 kernel for BitLinear causal self-attention (B=4, T=1024, C=1024, H=16).

Sharding: tensor-parallel over heads (2 heads/core, 8 cores) for qkv+attention,
then AllToAll reshards y to row (token) shards for the output projection.
qkv projection is interleaved with attention per batch sample; the AllToAll is
split in two by head-half so the first one overlaps attention. Per-core partial
layernorm stats for the second BitLinear ride inside the second AllToAll
payload (hi/lo bf16 split).
"""

import functools
import math
from contextlib import ExitStack

import ml_dtypes
import numpy as np

import concourse.bacc as bacc
import concourse.bass as bass
import concourse.mybir as mybir
import concourse.tile as tile
from concourse import masks as masks_mod
from concourse.bass_utils import run_bass_kernel_spmd

B, T, C = 4, 1024, 1024
H, HD = 16, 64
NCORES = 8
HPC = H // NCORES          # heads per core = 2
TOK = B * T                # 4096
RPC = TOK // NCORES        # token rows per core = 512
QB = 128.0
EPS = 1e-5

BF16 = mybir.dt.bfloat16
F32 = mybir.dt.float32
AF = mybir.ActivationFunctionType
ALU = mybir.AluOpType
AX = mybir.AxisListType


def _emit(nc, tc, ctx):
    qxT = nc.dram_tensor("qxT", [C, TOK], BF16, kind="ExternalInput")
    qwinT = nc.dram_tensor("qwinT", [C, 3 * HPC * HD], BF16, kind="ExternalInput")
    qwoutT = nc.dram_tensor("qwoutT", [C, C], BF16, kind="ExternalInput")
    consts = nc.dram_tensor("consts", [1, 8], F32, kind="ExternalInput")
    bsel = nc.dram_tensor("bsel", [1, 8], F32, kind="ExternalInput")
    msel = nc.dram_tensor("msel", [1, 8], F32, kind="ExternalInput")
    out = nc.dram_tensor("out", [RPC, C], F32, kind="ExternalOutput")

    singles = ctx.enter_context(tc.tile_pool(name="singles", bufs=1))
    big = ctx.enter_context(tc.tile_pool(name="big", bufs=3, space="PSUM"))
    small = ctx.enter_context(tc.tile_pool(name="small", bufs=2, space="PSUM"))
    sb = ctx.enter_context(tc.tile_pool(name="sb", bufs=2))
    dram = ctx.enter_context(tc.tile_pool(name="dram", bufs=1, space="DRAM"))

    # ---- setup ----
    ident_bf = singles.tile([128, 128], BF16)
    masks_mod.make_identity(nc, ident_bf[:])
    ident_f32 = singles.tile([128, 128], F32)
    masks_mod.make_identity(nc, ident_f32[:])

    ones_row = singles.tile([1, 128], F32)
    nc.vector.memset(ones_row[:], 1.0)
    ones_col = singles.tile([128, 1], F32)
    nc.vector.memset(ones_col[:], 1.0)
    ones8 = singles.tile([8, 1], F32)
    nc.vector.memset(ones8[:], 1.0)

    csb = singles.tile([1, 8], F32)
    nc.sync.dma_start(csb[:], consts[:])
    bsel_sb = singles.tile([1, 8], F32)
    nc.sync.dma_start(bsel_sb[:], bsel[:])
    msel_sb = singles.tile([1, 8], F32)
    nc.sync.dma_start(msel_sb[:], msel[:])

    cb_ps = small.tile([128, 8], F32, tag="small")
    nc.tensor.matmul(cb_ps[:], ones_row[:], csb[:])
    cbc = singles.tile([128, 8], F32)
    nc.vector.tensor_copy(cbc[:], cb_ps[:])

    qwin_all = singles.tile([128, 8 * 384], BF16)
    nc.sync.dma_start(qwin_all[:], qwinT.rearrange("(c p) o -> p c o", p=128))

    def qwin(c, lo, hi):
        return qwin_all[:, c * 384 + lo:c * 384 + hi]

    qT_sb = singles.tile([128, TOK], BF16)
    kT_sb = singles.tile([128, TOK], BF16)
    vT_sb = singles.tile([128, TOK], BF16)

    qxT_r = qxT.rearrange("(c p) t -> p c t", p=128)

    # A2A buffers: a2a1 blocks [64, 512] (head-local 0 rows); a2a2 blocks
    # [66, 512] (head-local 1 rows + stats hi/lo rows)
    a2a1_in = dram.tile([NCORES * 64, 512], BF16)
    a2a1_out = dram.tile([NCORES * 64, 512], BF16)
    a2a2_in = dram.tile([NCORES * 66, 512], BF16)
    a2a2_out = dram.tile([NCORES * 66, 512], BF16)
    a2a1_in_r = a2a1_in.rearrange("(bb h p) t -> p bb h t", p=64, h=2)
    a2a2_in_r = a2a2_in.rearrange("(j p) t -> p j t", p=66)

    yT_sb = singles.tile([128, TOK], BF16)
    stats = singles.tile([128, 9], F32)
    sq_tmp = singles.tile([128, 512], BF16)

    va = []
    for tb32 in range(32):
        t_ = singles.tile([128, 2 * (HD + 1)], BF16, tag=f"va{tb32}", name=f"va{tb32}")
        nc.vector.memset(t_[:, HD:HD + 1], 1.0)
        nc.vector.memset(t_[:, 2 * HD + 1:2 * HD + 2], 1.0)
        va.append(t_)

    def emit_qkv(b):
        """qkv projection + v transposes for t-blocks 2b, 2b+1 (tokens of batch b)."""
        for tb in (2 * b, 2 * b + 1):
            qx_tb = sb.tile([128, 8, 512], BF16, tag="qx", bufs=3, name=f"qx{tb}")
            nc.sync.dma_start(qx_tb[:], qxT_r[:, :, tb * 512:(tb + 1) * 512])
            qk_ps = big.tile([128, 1024], F32, tag="big", name=f"qkps{tb}")
            v_ps = big.tile([128, 512], F32, tag="big", name=f"vps{tb}")
            for c in range(8):
                st, sp = (c == 0), (c == 7)
                nc.tensor.matmul(qk_ps[:, 0:512], qwin(c, 0, 128), qx_tb[:, c, :], start=st, stop=sp)
                nc.tensor.matmul(qk_ps[:, 512:1024], qwin(c, 128, 256), qx_tb[:, c, :], start=st, stop=sp)
                nc.tensor.matmul(v_ps[:], qwin(c, 256, 384), qx_tb[:, c, :], start=st, stop=sp)
            nc.vector.tensor_copy(qT_sb[:, tb * 512:(tb + 1) * 512], qk_ps[:, 0:512])
            nc.vector.tensor_copy(kT_sb[:, tb * 512:(tb + 1) * 512], qk_ps[:, 512:1024])
            nc.vector.tensor_copy(vT_sb[:, tb * 512:(tb + 1) * 512], v_ps[:])
        for tb32 in range(8 * b, 8 * b + 8):
            tr_ps = small.tile([128, 128], BF16, tag="small", name=f"vtr{tb32}")
            nc.tensor.transpose(tr_ps[:], vT_sb[:, tb32 * 128:(tb32 + 1) * 128], ident_bf[:])
            nc.vector.tensor_copy(va[tb32][:, 0:HD], tr_ps[:, 0:HD])
            nc.vector.tensor_copy(va[tb32][:, HD + 1:2 * HD + 1], tr_ps[:, HD:2 * HD])

    def emit_pair(hl, b, pair_idx):
        qrow = hl * HD
        tbase = b * T
        se_tiles = {}
        for ib in range(2):
            jb_max = 4 * ib + 3
            for jp in range(0, (jb_max + 1) // 2):
                jb0, jb1 = 2 * jp, 2 * jp + 1
                s_ps = big.tile([128, 1024], F32, tag="big", name=f"s_ps{pair_idx}_{ib}_{jp}")
                for col, jb in ((0, jb0), (512, jb1)):
                    nc.tensor.matmul(
                        s_ps[:, col:col + 512],
                        kT_sb[qrow:qrow + HD, tbase + jb * 128:tbase + (jb + 1) * 128],
                        qT_sb[qrow:qrow + HD, tbase + ib * 512:tbase + (ib + 1) * 512],
                    )
                se = sb.tile([128, 1024], BF16, tag="se", bufs=12, name=f"se{pair_idx}_{ib}_{jp}")
                nc.scalar.activation(se[:], s_ps[:], AF.Exp, scale=cbc[:, 0:1])
                # causal mask: only the diagonal 128x128 sub-block is mixed;
                # fully-masked columns are never read by PV.
                for col, jb in ((0, jb0), (512, jb1)):
                    p = jb - 4 * ib
                    if 0 <= p <= 3:
                        dcol = col + 128 * p
                        nc.gpsimd.affine_select(
                            out=se[:, dcol:dcol + 128],
                            in_=se[:, dcol:dcol + 128],
                            compare_op=ALU.is_ge,
                            fill=0.0,
                            base=0,
                            pattern=[[1, 128]],
                            channel_multiplier=-1,
                        )
                se_tiles[(ib, jp)] = se
        y_pair = sb.tile([128, 512], BF16, tag="ypair", bufs=4, name=f"ypair{pair_idx}")
        for ib128 in range(8):
            ib512 = ib128 // 4
            icol = 128 * (ib128 % 4)
            pv_ps = small.tile([128, HD + 1], F32, tag="small", name=f"pv{pair_idx}_{ib128}")
            for jb in range(ib128 + 1):
                se = se_tiles[(ib512, jb // 2)]
                lhs = se[:, 512 * (jb % 2) + icol: 512 * (jb % 2) + icol + 128]
                nc.tensor.matmul(
                    pv_ps[:], lhs, va[b * 8 + jb][:, (HD + 1) * hl:(HD + 1) * hl + HD + 1],
                    start=(jb == 0), stop=(jb == ib128),
                )
            rec = sb.tile([128, 1], F32, tag="rec", bufs=2, name=f"rec{pair_idx}_{ib128}")
            nc.vector.reciprocal(rec[:], pv_ps[:, HD:HD + 1])
            nc.vector.tensor_scalar_mul(y_pair[:, ib128 * HD:(ib128 + 1) * HD], pv_ps[:, 0:HD], rec[:])
        for ib128 in range(8):
            ytr_ps = small.tile([HD, 128], BF16, tag="small", name=f"ytr{pair_idx}_{ib128}")
            nc.tensor.transpose(ytr_ps[:], y_pair[:, ib128 * HD:(ib128 + 1) * HD], ident_bf[:])
            nc.vector.tensor_copy(
                yT_sb[hl * HD:(hl + 1) * HD, tbase + ib128 * 128:tbase + (ib128 + 1) * 128],
                ytr_ps[:],
            )
        # stage this batch's y rows into the A2A input buffers
        if hl == 0:
            nc.sync.dma_start(a2a1_in_r[:, b, :, :], yT_sb[0:64, tbase:tbase + T])
        else:
            nc.sync.dma_start(a2a2_in_r[0:64, 2 * b:2 * b + 2, :], yT_sb[64:128, tbase:tbase + T])
        # stats
        s1 = sb.tile([128, 1], F32, tag="st1", bufs=2, name=f"s1_{pair_idx}")
        nc.vector.reduce_sum(s1[:], y_pair[:], axis=AX.X)
        s2 = sb.tile([128, 1], F32, tag="st2", bufs=2, name=f"s2_{pair_idx}")
        nc.scalar.activation(sq_tmp[:], y_pair[:], AF.Square, accum_out=s2[:])
        s3 = sb.tile([128, 1], F32, tag="st3", bufs=2, name=f"s3_{pair_idx}")
        nc.vector.reduce_max(s3[:], y_pair[:], axis=AX.X, apply_absolute_value=True)
        if hl == 0:
            nc.vector.tensor_copy(stats[:, b:b + 1], s1[:])
            nc.vector.tensor_copy(stats[:, 4 + b:5 + b], s2[:])
        else:
            nc.vector.tensor_add(stats[:, b:b + 1], stats[:, b:b + 1], s1[:])
            nc.vector.tensor_add(stats[:, 4 + b:5 + b], stats[:, 4 + b:5 + b], s2[:])
        if pair_idx == 0:
            nc.vector.tensor_copy(stats[:, 8:9], s3[:])
        else:
            nc.vector.tensor_max(stats[:, 8:9], stats[:, 8:9], s3[:])

    # ---- interleaved schedule ----
    emit_qkv(0)
    emit_pair(0, 0, 0)
    emit_pair(1, 0, 1)
    pi = 2
    for b in range(1, 4):
        emit_qkv(b)
        emit_pair(0, b, pi)
        pi += 1
    nc.gpsimd.collective_compute(
        "AllToAll", ALU.bypass, replica_groups=[list(range(NCORES))],
        ins=[a2a1_in.opt()], outs=[a2a1_out.opt()],
    )
    # prefetch output-proj weights + first half of resharded y while attention finishes
    qwout_all = singles.tile([128, 8 * 1024], BF16)
    nc.sync.dma_start(qwout_all[:], qwoutT.rearrange("(c p) o -> p c o", p=128))
    qy = singles.tile([128, 8, 512], BF16)
    a2a1_o_r = a2a1_out.rearrange("(j p) t -> p j t", p=64)
    a2a2_o_r = a2a2_out.rearrange("(j p) t -> p j t", p=66)
    nc.sync.dma_start(qy[0:64, :, :], a2a1_o_r[:, :, :])

    def qwout(c, lo, hi):
        return qwout_all[:, c * 1024 + lo:c * 1024 + hi]

    for b in range(1, 4):
        emit_pair(1, b, pi)
        pi += 1

    # ---- stats partition-reduce, stage stats rows, second A2A ----
    st_ps = small.tile([1, 9], F32, tag="small")
    nc.tensor.matmul(st_ps[:], ones_col[:], stats[:])
    trm_ps = small.tile([1, 128], F32, tag="small")
    nc.tensor.transpose(trm_ps[:], stats[:, 8:9], ident_f32[:])
    gmax_l = singles.tile([1, 1], F32)
    nc.vector.reduce_max(gmax_l[:], trm_ps[:], axis=AX.X)

    srow = singles.tile([1, 512], F32)
    nc.vector.memset(srow[:], 0.0)
    nc.vector.tensor_copy(srow[:, 0:8], st_ps[:, 0:8])
    nc.vector.tensor_scalar_mul(srow[:, 8:16], msel_sb[:], gmax_l[:])
    srep_ps = small.tile([8, 512], F32, tag="small")
    nc.tensor.matmul(srep_ps[:], ones_row[:, 0:8], srow[:])
    srep = singles.tile([8, 512], F32)
    nc.vector.tensor_copy(srep[:], srep_ps[:])
    hi8 = singles.tile([8, 512], BF16)
    nc.vector.tensor_copy(hi8[:], srep[:])
    hi8f = singles.tile([8, 512], F32)
    nc.vector.tensor_copy(hi8f[:], hi8[:])
    lo8 = singles.tile([8, 512], BF16)
    nc.vector.tensor_sub(lo8[:], srep[:], hi8f[:])
    nc.sync.dma_start(a2a2_in_r[64, :, :], hi8[:])
    nc.sync.dma_start(a2a2_in_r[65, :, :], lo8[:])
    nc.gpsimd.collective_compute(
        "AllToAll", ALU.bypass, replica_groups=[list(range(NCORES))],
        ins=[a2a2_in.opt()], outs=[a2a2_out.opt()],
    )

    # ---- global stats, quantize, output projection ----
    nc.sync.dma_start(qy[64:128, :, :], a2a2_o_r[0:64, :, :])
    sr_hi = singles.tile([8, 16], BF16)
    nc.sync.dma_start(sr_hi[:], a2a2_o_r[64, :, 0:16])
    sr_lo = singles.tile([8, 16], BF16)
    nc.sync.dma_start(sr_lo[:], a2a2_o_r[65, :, 0:16])
    stats_f = singles.tile([8, 16], F32)
    nc.vector.tensor_add(stats_f[:], sr_hi[:], sr_lo[:])

    glob_ps = small.tile([1, 16], F32, tag="small")
    nc.tensor.matmul(glob_ps[:], ones8[:], stats_f[:])
    sc = singles.tile([1, 24], F32)  # mu 0:4, msq 4:8, var 8:12, A 12:16, B 16:20
    inv_tc = 1.0 / float(T * C)
    nc.vector.tensor_scalar_mul(sc[:, 0:8], glob_ps[:, 0:8], inv_tc)
    gmax = singles.tile([1, 1], F32)
    nc.vector.reduce_max(gmax[:], glob_ps[:, 8:16], axis=AX.X)
    nc.vector.tensor_mul(sc[:, 8:12], sc[:, 0:4], sc[:, 0:4])
    nc.vector.tensor_sub(sc[:, 8:12], sc[:, 4:8], sc[:, 8:12])
    nc.vector.tensor_scalar_add(sc[:, 8:12], sc[:, 8:12], 1e-5)
    sig = singles.tile([1, 4], F32)
    nc.scalar.activation(sig[:], sc[:, 8:12], AF.Sqrt)
    rsig = singles.tile([1, 4], F32)
    nc.vector.reciprocal(rsig[:], sig[:])
    nc.vector.tensor_scalar_mul(sc[:, 12:16], rsig[:], csb[:, 1:2])
    nc.vector.tensor_mul(sc[:, 16:20], sc[:, 0:4], sc[:, 12:16])
    # select this core's batch scalars with one 2-group reduce over (A4|B4)*bsel8
    tsel = singles.tile([1, 8], F32)
    nc.vector.tensor_mul(tsel[:], sc[:, 12:20], bsel_sb[:])
    row4 = singles.tile([1, 4], F32)
    nc.vector.reduce_sum(row4[:, 0:2], tsel.rearrange("p (g f) -> p g f", g=2), axis=AX.X)
    nc.vector.tensor_scalar_mul(row4[:, 3:4], gmax[:], csb[:, 2:3])
    nc.vector.tensor_scalar_mul(row4[:, 2:3], row4[:, 3:4], -1.0)
    qsc_ps = small.tile([128, 4], F32, tag="small")
    nc.tensor.matmul(qsc_ps[:], ones_row[:], row4[:])
    qsc = singles.tile([128, 4], F32)
    nc.vector.tensor_copy(qsc[:], qsc_ps[:])

    qy_flat = qy.rearrange("p j t -> p (j t)")
    nc.vector.tensor_scalar(
        out=qy_flat, in0=qy_flat, scalar1=qsc[:, 0:1], scalar2=qsc[:, 1:2],
        op0=ALU.mult, op1=ALU.subtract,
    )
    nc.vector.tensor_scalar(
        out=qy_flat, in0=qy_flat, scalar1=qsc[:, 2:3], scalar2=qsc[:, 3:4],
        op0=ALU.max, op1=ALU.min,
    )

    for tch in range(4):
        osb = sb.tile([128, 1024], F32, tag="ob", bufs=2, name=f"osb{tch}")
        for oh in range(2):
            o_ps = big.tile([128, 512], F32, tag="big", name=f"ops{tch}_{oh}")
            for cj in range(8):
                nc.tensor.matmul(
                    o_ps[:], qy[:, cj, tch * 128:(tch + 1) * 128],
                    qwout(cj, oh * 512, (oh + 1) * 512),
                    start=(cj == 0), stop=(cj == 7),
                )
            nc.vector.tensor_copy(osb[:, oh * 512:(oh + 1) * 512], o_ps[:])
        nc.sync.dma_start(out[tch * 128:(tch + 1) * 128, :], osb[:])


@functools.lru_cache(maxsize=1)
def build():
    nc = bacc.Bacc(None)
    with tile.TileContext(nc) as tc:
        with ExitStack() as ctx:
            _emit(nc, tc, ctx)
    nc.finalize()
    return nc


def _host_prep(x, w_in, w_out):
    x = np.asarray(x, np.float32)
    w_in = np.asarray(w_in, np.float32)
    w_out = np.asarray(w_out, np.float32)

    a1 = w_in.mean()
    qw1 = np.sign(w_in - a1).astype(np.float32)
    b1 = np.abs(w_in).mean()
    a2 = w_out.mean()
    qw2 = np.sign(w_out - a2).astype(np.float32)
    b2 = np.abs(w_out).mean()

    mu = x.mean(axis=(1, 2), keepdims=True)
    var = x.var(axis=(1, 2), keepdims=True)
    g1 = np.abs(x).max()
    xn = (x - mu) / np.sqrt(var + 1e-5)
    qx = np.clip(xn * (QB / g1), -QB + EPS, QB - EPS)
    scale1 = b1 * g1 / QB

    bf = ml_dtypes.bfloat16
    qxT = np.ascontiguousarray(qx.reshape(TOK, C).T).astype(bf)
    qwoutT = np.ascontiguousarray(qw2.T).astype(bf)
    att_scale = scale1 * scale1 / math.sqrt(HD)
    cbound = (QB - EPS) / QB * b2 * scale1
    consts = np.array([[att_scale, b2, cbound, 0, 0, 0, 0, 0]], np.float32)

    in_maps = []
    for core in range(NCORES):
        r0 = core * 128
        qwin = np.concatenate(
            [qw1[r0:r0 + 128], qw1[C + r0:C + r0 + 128], qw1[2 * C + r0:2 * C + r0 + 128]], axis=0
        )
        qwinT = np.ascontiguousarray(qwin.T).astype(bf)
        bsel_ = np.zeros((1, 8), np.float32)
        bsel_[0, core // 2] = 1.0
        bsel_[0, 4 + core // 2] = 1.0
        msel_ = np.zeros((1, 8), np.float32)
        msel_[0, core] = 1.0
        in_maps.append({
            "qxT": qxT, "qwinT": qwinT, "qwoutT": qwoutT,
            "consts": consts, "bsel": bsel_, "msel": msel_,
        })
    return in_maps


def kernel(x, w_in, w_out):
    in_maps = _host_prep(x, w_in, w_out)
    nc = build()
    res = run_bass_kernel_spmd(nc, in_maps, core_ids=list(range(NCORES)))
    out = np.concatenate([np.asarray(res.results[i]["out"]) for i in range(NCORES)], axis=0)
    return out.reshape(B, T, C).astype(np.float32)
